# revision 1
# baseline (speedup 1.0000x reference)
"""Trainium2 Bass kernel for nn_DYS_opt_net: MLP + 50-iteration DYS fixed-point
loop + one final differentiable step, data-parallel over the batch on 8 cores.

Math (equivalent to the SVD-projector reference):
  P = pinv(A) A = V V^T with V = R^T, R = L^{-1} A, L = chol(A A^T)  (host, fp64)
  c = pinv(A) b = R^T (L^{-1} b)
  w = relu(d W1 + b1) W2 + b2
  51 identical updates:  x = relu(z); u = 2x - z - a*w
                         z' = x - a*w + c - (u V) V^T
  output = relu(z_51)

Device strategy (per core, batch slice of 32 rows, N2-major layout):
  state z^T stored as (128, 32, 32) tiles (partition = N2 % 128).
  mm1:  dT^T = V^T-chunks (stationary fp16 V tiles) x u^T (moving fp16), out
        T^T (1024-major) in PSUM — no transposes anywhere in the loop.
  mm2:  Z2^T = VT-chunks stationary x T^T moving, split precision:
        hi pass in fp16 + lo pass in scaled fp8 (VTlo*2^16, T*2^5) to recover
        ~fp32 projector accuracy (dominant error source is VT quantization).
"""

import os
import sys
import math
from contextlib import ExitStack

import numpy as np
import ml_dtypes

try:
    import concourse.bass as bass
except ImportError:
    sys.path.insert(0, "/opt/trn_rl_repo")
    import concourse.bass as bass

import concourse.tile as tile
from concourse import bacc, mybir
from concourse.bass_utils import run_bass_kernel_spmd

F16 = mybir.dt.float16
F8 = mybir.dt.float8e4
F32 = mybir.dt.float32
NP_F16 = np.float16
NP_F8 = ml_dtypes.float8_e4m3fn

ALPHA = np.float32(0.05)
NCORES = 8
B, D, H, N1, N2 = 256, 512, 2048, 1024, 4096
BS = B // NCORES          # 32 batch rows per core
ITERS = int(os.environ.get("DYS_ITERS", "51"))   # 50 loop + 1 final step
UNROLL_STATIC = os.environ.get("DYS_UNROLL", "0") == "1"
LO_SCALE_V = 16           # VTlo stored * 2^16
LO_SCALE_T = 5            # T cast to fp8 * 2^5
LO_UNSCALE = float(2.0 ** (-(LO_SCALE_V + LO_SCALE_T)))

def _dt_np(x, dt):
    return np.ascontiguousarray(x, dtype=dt)


def _build_program():
    """Build + tile-schedule the SPMD program (same for every core)."""
    nc = bacc.Bacc("TRN2", target_bir_lowering=False, debug=False,
                   num_devices=NCORES)

    # ---- DRAM parameters (per-core layouts; host pre-arranges) ----
    dram = {}
    def din(name, shape, dt):
        dram[name] = nc.dram_tensor(name, list(shape), dt, kind="ExternalInput").ap()
    din("v16", (128, N2 // 128, N1), F16)            # V n2-major
    din("vthi", (128, N1 // 128, N2), F16)           # V^T rank-major (hi)
    din("vtlo8", (128, N2 // 128, N1 // 128, 128), F8)  # V^T lo, m-major tiles
    din("w1t", (128, H // 128, D // 128, 128), F16)  # W1 d-major, m-major tiles
    din("w2t", (128, N2 // 128, H // 128, 128), F16)  # W2 h-major, m-major tiles
    din("dt16", (128, D // 128, BS), F16)            # d^T slice
    din("z0t", (128, N2 // 128, BS), F32)            # z0^T slice
    din("cvec", (128, N2 // 128), F32)               # c
    din("ab2", (128, N2 // 128), F32)                # alpha*b2
    din("b1t", (128, H // 128), F32)                 # b1
    out_d = nc.dram_tensor("outt", [128, N2 // 128, BS], F32, kind="ExternalOutput").ap()

    KD, KH, KN2, KN1 = D // 128, H // 128, N2 // 128, N1 // 128   # 4, 16, 32, 8

    with tile.TileContext(nc) as tc:
        with ExitStack() as ctx:
            res = ctx.enter_context(tc.tile_pool(name="resident", bufs=1))
            st = ctx.enter_context(tc.tile_pool(name="state", bufs=1))
            tmp = ctx.enter_context(tc.tile_pool(name="tmp", bufs=1))
            w2p = ctx.enter_context(tc.tile_pool(name="w2s", bufs=3))
            ps = ctx.enter_context(tc.tile_pool(name="ps", bufs=1, space="PSUM"))

            # ---- resident weights ----
            v16 = res.tile([128, KN2, N1], F16)
            vthi = res.tile([128, KN1, N2], F16)
            vlo8 = res.tile([128, KN2, KN1, 128], F8)
            nc.sync.dma_start(v16[:], dram["v16"][:])
            nc.sync.dma_start(vthi[:], dram["vthi"][:])
            nc.sync.dma_start(vlo8[:], dram["vtlo8"][:])

            # ---- state ----
            zt = st.tile([128, KN2, BS], F32)
            aw = st.tile([128, KN2, BS], F32)
            cv = st.tile([128, KN2], F32)
            ab2 = st.tile([128, KN2], F32)
            u16 = st.tile([128, KN2, BS], F16)
            tt16 = st.tile([128, KN1, BS], F16)
            tt8 = st.tile([128, KN1, BS], F8)
            nc.sync.dma_start(zt[:], dram["z0t"][:])
            nc.sync.dma_start(cv[:], dram["cvec"][:])
            nc.sync.dma_start(ab2[:], dram["ab2"][:])

            # ---- MLP: w^T = W2^T relu(W1^T d^T + b1) + b2, then aw = a*w^T ----
            dt16 = st.tile([128, KD, BS], F16)
            b1t = st.tile([128, KH], F32)
            nc.sync.dma_start(dt16[:], dram["dt16"][:])
            nc.sync.dma_start(b1t[:], dram["b1t"][:])

            ht_ps = ps.tile([128, KH, BS], F32)
            for m in range(KH):
                w1tile = w2p.tile([128, KD, 128], F16, tag="w1t")
                nc.sync.dma_start(w1tile[:], dram["w1t"][:, m])
                for k in range(KD):
                    nc.tensor.matmul(ht_ps[:, m, :], w1tile[:, k, :],
                                     dt16[:, k, :], start=(k == 0), stop=(k == KD - 1))
            hadd = tmp.tile([128, KH, BS], F32, tag="t")
            nc.vector.tensor_add(hadd[:], ht_ps[:], b1t[:, :, None].to_broadcast((128, KH, BS)))
            ht16 = st.tile([128, KH, BS], F16)
            nc.scalar.activation(ht16[:], hadd[:], mybir.ActivationFunctionType.Relu)

            wt_ps = ps.tile([128, KN2, BS], F32)
            for m in range(KN2):
                w2tile = w2p.tile([128, KH, 128], F16, tag="w2t")
                nc.sync.dma_start(w2tile[:], dram["w2t"][:, m])
                for k in range(KH):
                    nc.tensor.matmul(wt_ps[:, m, :], w2tile[:, k, :], ht16[:, k, :],
                                     start=(k == 0), stop=(k == KH - 1))
            nc.vector.tensor_scalar_mul(aw[:], wt_ps[:], float(ALPHA))
            nc.vector.tensor_add(aw[:], aw[:], ab2[:, :, None].to_broadcast((128, KN2, BS)))

            # ---- DYS iterations ----
            tt_ps = ps.tile([128, KN1, BS], F32)
            z2hi_ps = ps.tile([128, KN2, BS], F32)
            z2lo_ps = ps.tile([128, KN2, BS], F32)

            MG = 2                       # zn drained per PSUM bank (16 chunks)
            MGS = KN2 // MG

            q = tmp.tile([128, KN2, BS], F32, tag="q")
            t = tmp.tile([128, KN2, BS], F32, tag="t")
            s = tmp.tile([128, KN2, BS], F32, tag="s")
            lo = tmp.tile([128, KN2, BS], F32, tag="lo")

            def prep_chain(msl):
                """From fresh zt[msl]: u16 and s for the next iteration."""
                nc.scalar.activation(q[:, msl], zt[:, msl],
                                     mybir.ActivationFunctionType.Relu, scale=2.0)
                nc.vector.tensor_sub(t[:, msl], q[:, msl], zt[:, msl])
                nc.vector.tensor_sub(u16[:, msl], t[:, msl], aw[:, msl])
                nc.scalar.activation(s[:, msl], q[:, msl],
                                     mybir.ActivationFunctionType.Copy, scale=0.5)
                nc.vector.tensor_sub(s[:, msl], s[:, msl], aw[:, msl])
                nc.vector.tensor_add(s[:, msl], s[:, msl],
                                     cv[:, msl, None].to_broadcast((128, msl.stop - msl.start, BS)))

            prep_chain(slice(0, KN2))    # prologue for iteration 0

            def body(_i):
                # mm1: T^T += V-chunk^T @ u^T (stationary V fp16)
                for m in range(KN1):
                    for k in range(KN2):
                        nc.tensor.matmul(tt_ps[:, m, :],
                                         v16[:, k, m * 128:(m + 1) * 128],
                                         u16[:, k, :],
                                         start=(k == 0), stop=(k == KN2 - 1))
                nc.scalar.activation(tt16[:], tt_ps[:], mybir.ActivationFunctionType.Copy)
                nc.scalar.activation(tt8[:], tt_ps[:], mybir.ActivationFunctionType.Copy,
                                     scale=float(2.0 ** LO_SCALE_T))
                # mm2 hi/lo + per-bank drain; the drain also prepares the next
                # iteration's u16/s so the chain overlaps mm2 of the next group
                for mg in range(MG):
                    for m in range(mg * MGS, (mg + 1) * MGS):
                        for k in range(KN1):
                            nc.tensor.matmul(z2hi_ps[:, m, :],
                                             vthi[:, k, m * 128:(m + 1) * 128],
                                             tt16[:, k, :],
                                             start=(k == 0), stop=(k == KN1 - 1))
                        for k in range(KN1):
                            nc.tensor.matmul(z2lo_ps[:, m, :], vlo8[:, m, k, :],
                                             tt8[:, k, :],
                                             start=(k == 0), stop=(k == KN1 - 1))
                    msl = slice(mg * MGS, (mg + 1) * MGS)
                    # z' = s - Z2hi - Z2lo * 2^-21 for this group
                    nc.scalar.activation(lo[:, msl], z2lo_ps[:, msl],
                                         mybir.ActivationFunctionType.Copy,
                                         scale=LO_UNSCALE)
                    nc.vector.tensor_sub(t[:, msl], s[:, msl], z2hi_ps[:, msl])
                    nc.vector.tensor_sub(zt[:, msl], t[:, msl], lo[:, msl])
                    prep_chain(msl)

            if UNROLL_STATIC:
                for i in range(ITERS):
                    body(i)
            else:
                with tc.For_i(0, ITERS, 1, hint_engines=(mybir.EngineType.PE,)) as i:
                    body(i)

            # ---- output: relu(z) ----
            outs = tmp.tile([128, KN2, BS], F32, tag="q")
            nc.scalar.activation(outs[:], zt[:], mybir.ActivationFunctionType.Relu)
            nc.sync.dma_start(out_d[:], outs[:])

    nc.compile()
    return nc


_CACHE = {}


def _host_factors(A, b_vec):
    A64 = A.astype(np.float64)
    L = np.linalg.cholesky(A64 @ A64.T)
    R = np.linalg.solve(L, A64)                     # (N1, N2), orthonormal rows
    q = np.linalg.solve(L, b_vec.astype(np.float64))
    c = (R.T @ q).astype(np.float32)                # (N2,)
    VT = R.astype(np.float32)                       # (N1, N2) = V^T
    V = np.ascontiguousarray(VT.T)                  # (N2, N1)
    return V, VT, c


def host_in_maps(d, A, b_vec, W1, b1, W2, b2, z0):
    """Host-side factor computation + per-core DRAM layouts."""
    V, VT, c = _host_factors(A, b_vec)

    v16 = _dt_np(V.reshape(N2 // 128, 128, N1).transpose(1, 0, 2), NP_F16)
    vthi_f = VT.astype(NP_F16)
    vthi = _dt_np(vthi_f.reshape(N1 // 128, 128, N2).transpose(1, 0, 2), NP_F16)
    vtlo = (VT - vthi_f.astype(np.float32)) * np.float32(2.0 ** LO_SCALE_V)
    # (N1, N2) -> (128, m=N2/128, k=N1/128, 128)
    vtlo8 = _dt_np(
        vtlo.reshape(N1 // 128, 128, N2 // 128, 128).transpose(1, 2, 0, 3), NP_F8)
    w1t = _dt_np(
        W1.astype(NP_F16).reshape(D // 128, 128, H // 128, 128).transpose(1, 2, 0, 3),
        NP_F16)
    w2t = _dt_np(
        W2.astype(NP_F16).reshape(H // 128, 128, N2 // 128, 128).transpose(1, 2, 0, 3),
        NP_F16)
    cvec = _dt_np(c.reshape(N2 // 128, 128).T, np.float32)
    ab2 = _dt_np((ALPHA * b2.astype(np.float32)).reshape(N2 // 128, 128).T, np.float32)
    b1t = _dt_np(b1.astype(np.float32).reshape(H // 128, 128).T, np.float32)

    shared = {"v16": v16, "vthi": vthi, "vtlo8": vtlo8, "w1t": w1t, "w2t": w2t,
              "cvec": cvec, "ab2": ab2, "b1t": b1t}

    in_maps = []
    for i in range(NCORES):
        rows = slice(i * BS, (i + 1) * BS)
        dT = np.ascontiguousarray(d[rows].T)        # (D, BS)
        dt16 = _dt_np(dT.reshape(D // 128, 128, BS).transpose(1, 0, 2), NP_F16)
        z0T = np.ascontiguousarray(z0[rows].T)      # (N2, BS)
        z0t = _dt_np(z0T.reshape(N2 // 128, 128, BS).transpose(1, 0, 2), np.float32)
        in_maps.append({**shared, "dt16": dt16, "z0t": z0t})
    return in_maps


def kernel(d, A, b_vec, W1, b1, W2, b2, z0):
    in_maps = host_in_maps(d, A, b_vec, W1, b1, W2, b2, z0)

    if "nc" not in _CACHE:
        _CACHE["nc"] = _build_program()
    nc = _CACHE["nc"]

    trace = os.environ.get("DYS_TRACE", "0") == "1"
    res = run_bass_kernel_spmd(nc, in_maps, list(range(NCORES)), trace=trace)
    _CACHE["last_result"] = res

    out = np.empty((B, N2), dtype=np.float32)
    for i in range(NCORES):
        arr = res.results[i]["outt"]                # (128, N2/128, BS)
        out[i * BS:(i + 1) * BS] = arr.transpose(2, 1, 0).reshape(BS, N2)
    return out



# revision 5
# speedup vs baseline: 1.5566x; 1.5566x over previous
"""Trainium2 Bass kernel for nn_DYS_opt_net: MLP + 50-iteration DYS fixed-point
loop + one final differentiable step, data-parallel over the batch on 8 cores.

Math (equivalent to the SVD-projector reference):
  P = pinv(A) A = V V^T with V = R^T, R = L^{-1} A, L = chol(A A^T)  (host, fp64)
  c = pinv(A) b = R^T (L^{-1} b)
  w = relu(d W1 + b1) W2 + b2
  51 identical updates:  x = relu(z); u = 2x - z - a*w = |z| - a*w
                         z' = x - a*w + c - (u V) V^T = s - (u V) V^T
  with s = relu(z) + sm, sm = c - a*w (constant across iterations).
  output = relu(z_51)

Device strategy (per core, batch slice of 32 rows, N2-major layout):
  state z^T stored as (128, 32, 32) tiles (partition = N2 % 128).
  mm1:  T^T = V-chunk^T x u^T, k-outer so matmuls consume u16 chunks in the
        order the previous iteration's prep produced them (no boundary stall).
  mm2:  z2^T = V^T-chunks x T^T in fp16 (single pass; fp16 projector error
        ~2e-3 final, well inside the 2e-2 gate), 4 m-groups with alternating
        PSUM tiles; each group's drain + next-iter prep overlaps the next
        group's matmuls on vector/scalar engines.
"""

import os
import sys
from contextlib import ExitStack

import numpy as np

try:
    import concourse.bass as bass
except ImportError:
    sys.path.insert(0, "/opt/trn_rl_repo")
    import concourse.bass as bass

import concourse.tile as tile
from concourse import bacc, mybir
from concourse.bass_utils import run_bass_kernel_spmd

F16 = mybir.dt.float16
F32 = mybir.dt.float32
NP_F16 = np.float16

ALPHA = np.float32(0.05)
NCORES = 8
B, D, H, N1, N2 = 256, 512, 2048, 1024, 4096
BS = B // NCORES          # 32 batch rows per core
ITERS = int(os.environ.get("DYS_ITERS", "51"))   # 50 loop + 1 final step
UNROLL_STATIC = os.environ.get("DYS_UNROLL", "0") == "1"

KD, KH, KN2, KN1 = D // 128, H // 128, N2 // 128, N1 // 128   # 4, 16, 32, 8
G = 4                     # mm2 drain groups
GM = KN2 // G             # m-chunks per group (8)


def _dt_np(x, dt):
    return np.ascontiguousarray(x, dtype=dt)


def _build_program():
    """Build + tile-schedule the SPMD program (same for every core)."""
    nc = bacc.Bacc("TRN2", target_bir_lowering=False, debug=False,
                   num_devices=NCORES)

    dram = {}
    def din(name, shape, dt):
        dram[name] = nc.dram_tensor(name, list(shape), dt, kind="ExternalInput").ap()
    din("dt16", (128, KD, BS), F16)              # d^T slice
    din("b1t", (128, KH), F32)                   # b1
    din("z0t", (128, KN2, BS), F32)              # z0^T slice
    din("cvec", (128, KN2), F32)                 # c
    din("ab2", (128, KN2), F32)                  # alpha*b2
    din("w1t", (128, KH, KD, 128), F16)          # W1 d-major, m-major tiles
    din("w2t", (128, KN2, KH, 128), F16)         # W2 h-major, m-major tiles
    din("v16", (128, KN2, N1), F16)              # V n2-major
    din("vtm", (128, KN2, KN1, 128), F16)        # V^T, m-major tiles
    out_d = nc.dram_tensor("outt", [128, KN2, BS], F32, kind="ExternalOutput").ap()

    with tile.TileContext(nc) as tc:
        with ExitStack() as ctx:
            res = ctx.enter_context(tc.tile_pool(name="resident", bufs=1))
            st = ctx.enter_context(tc.tile_pool(name="state", bufs=1))
            tmp = ctx.enter_context(tc.tile_pool(name="tmp", bufs=1))
            w2p = ctx.enter_context(tc.tile_pool(name="w2s", bufs=3))
            ps = ctx.enter_context(tc.tile_pool(name="ps", bufs=1, space="PSUM"))

            # ---- small front-loaded DMAs (everything the MLP + prologue needs) ----
            dt16 = st.tile([128, KD, BS], F16)
            b1t = st.tile([128, KH], F32)
            zt = st.tile([128, KN2, BS], F32)
            cv = st.tile([128, KN2], F32)
            ab2 = st.tile([128, KN2], F32)
            nc.sync.dma_start(dt16[:], dram["dt16"][:])
            nc.sync.dma_start(b1t[:], dram["b1t"][:])
            nc.sync.dma_start(zt[:], dram["z0t"][:])
            nc.sync.dma_start(cv[:], dram["cvec"][:])
            nc.sync.dma_start(ab2[:], dram["ab2"][:])

            # ---- MLP: w^T = W2^T relu(W1^T d^T + b1) + b2, then aw = a*w^T ----
            ht_ps = ps.tile([128, KH, BS], F32)
            for m in range(KH):
                w1tile = w2p.tile([128, KD, 128], F16, tag="w1t")
                nc.sync.dma_start(w1tile[:], dram["w1t"][:, m])
                for k in range(KD):
                    nc.tensor.matmul(ht_ps[:, m, :], w1tile[:, k, :],
                                     dt16[:, k, :], start=(k == 0), stop=(k == KD - 1))
            hadd = tmp.tile([128, KH, BS], F32, tag="a")
            nc.vector.tensor_add(hadd[:], ht_ps[:], b1t[:, :, None].to_broadcast((128, KH, BS)))
            ht16 = st.tile([128, KH, BS], F16)
            nc.scalar.activation(ht16[:], hadd[:], mybir.ActivationFunctionType.Relu)

            wt_ps = ps.tile([128, KN2, BS], F32)
            for m in range(KN2):
                w2tile = w2p.tile([128, KH, 128], F16, tag="w2t")
                nc.sync.dma_start(w2tile[:], dram["w2t"][:, m])
                for k in range(KH):
                    nc.tensor.matmul(wt_ps[:, m, :], w2tile[:, k, :], ht16[:, k, :],
                                     start=(k == 0), stop=(k == KH - 1))
            aw = st.tile([128, KN2, BS], F32)
            sm = st.tile([128, KN2, BS], F32)
            nc.vector.tensor_scalar_mul(aw[:], wt_ps[:], float(ALPHA))
            nc.vector.tensor_add(aw[:], aw[:], ab2[:, :, None].to_broadcast((128, KN2, BS)))
            # sm = c - aw
            nc.vector.tensor_scalar_mul(sm[:], aw[:], -1.0)
            nc.vector.tensor_add(sm[:], sm[:], cv[:, :, None].to_broadcast((128, KN2, BS)))

            # ---- resident V weights (queued after MLP weights; loop chases) ----
            v16 = res.tile([128, KN2, N1], F16)
            vtm = res.tile([128, KN2, KN1, 128], F16)
            for c8 in range(0, KN2, 8):
                nc.sync.dma_start(v16[:, c8:c8 + 8, :], dram["v16"][:, c8:c8 + 8, :])
            for c8 in range(0, KN2, 8):
                nc.sync.dma_start(vtm[:, c8:c8 + 8], dram["vtm"][:, c8:c8 + 8])

            # ---- state + prologue prep from z0 ----
            u16 = st.tile([128, KN2, BS], F16)
            s = st.tile([128, KN2, BS], F32)
            tt16 = st.tile([128, KN1, BS], F16)
            ab_t = tmp.tile([128, KN2, BS], F32, tag="a")
            rl_t = tmp.tile([128, KN2, BS], F32, tag="r")

            def prep(gsl):
                """From fresh zt[gsl]: u16 = |z| - aw, s = relu(z) + sm."""
                n = gsl.stop - gsl.start
                nc.scalar.activation(ab_t[:, gsl], zt[:, gsl],
                                     mybir.ActivationFunctionType.Abs)
                nc.vector.tensor_sub(u16[:, gsl], ab_t[:, gsl], aw[:, gsl])
                nc.scalar.activation(rl_t[:, gsl], zt[:, gsl],
                                     mybir.ActivationFunctionType.Relu)
                nc.vector.tensor_add(s[:, gsl], rl_t[:, gsl], sm[:, gsl])

            prep(slice(0, KN2))

            # ---- DYS iterations ----
            tt_ps = ps.tile([128, KN1, BS], F32)
            z2a = ps.tile([128, GM, BS], F32)
            z2b = ps.tile([128, GM, BS], F32)
            z2ps = [z2a, z2b]

            def body(_i):
                # mm1: T^T += V-chunk^T @ u^T (one PSUM group open per bank:
                # m-outer, k-inner; first m-chunk consumes u16 in k order)
                for m in range(KN1):
                    for k in range(KN2):
                        nc.tensor.matmul(tt_ps[:, m, :],
                                         v16[:, k, m * 128:(m + 1) * 128],
                                         u16[:, k, :],
                                         start=(k == 0), stop=(k == KN2 - 1))
                # copy T to fp16 SBUF, split across scalar+vector engines
                nc.scalar.activation(tt16[:, 0:4], tt_ps[:, 0:4],
                                     mybir.ActivationFunctionType.Copy)
                nc.vector.tensor_scalar_mul(tt16[:, 4:8], tt_ps[:, 4:8], 1.0)
                # mm2 in G groups; drain + next-iter prep overlap next group
                for g in range(G):
                    zp = z2ps[g % 2]
                    base = g * GM
                    for mo in range(GM):
                        for k in range(KN1):
                            nc.tensor.matmul(zp[:, mo, :],
                                             vtm[:, base + mo, k, :],
                                             tt16[:, k, :],
                                             start=(k == 0), stop=(k == KN1 - 1))
                    gsl = slice(base, base + GM)
                    nc.vector.tensor_sub(zt[:, gsl], s[:, gsl], zp[:])
                    prep(gsl)

            if UNROLL_STATIC:
                for i in range(ITERS):
                    body(i)
            else:
                with tc.For_i(0, ITERS, 1, hint_engines=(mybir.EngineType.PE,)) as i:
                    body(i)

            # ---- output: relu(z) ----
            outs = tmp.tile([128, KN2, BS], F32, tag="r")
            nc.scalar.activation(outs[:], zt[:], mybir.ActivationFunctionType.Relu)
            nc.sync.dma_start(out_d[:], outs[:])

    nc.compile()
    return nc


_CACHE = {}


def _host_factors(A, b_vec):
    A64 = A.astype(np.float64)
    L = np.linalg.cholesky(A64 @ A64.T)
    R = np.linalg.solve(L, A64)                     # (N1, N2), orthonormal rows
    q = np.linalg.solve(L, b_vec.astype(np.float64))
    c = (R.T @ q).astype(np.float32)                # (N2,)
    VT = R.astype(np.float32)                       # (N1, N2) = V^T
    V = np.ascontiguousarray(VT.T)                  # (N2, N1)
    return V, VT, c


def host_in_maps(d, A, b_vec, W1, b1, W2, b2, z0):
    """Host-side factor computation + per-core DRAM layouts."""
    V, VT, c = _host_factors(A, b_vec)

    v16 = _dt_np(V.reshape(KN2, 128, N1).transpose(1, 0, 2), NP_F16)
    # (N1, N2) -> (128, m=N2/128, k=N1/128, 128)
    vtm = _dt_np(
        VT.astype(NP_F16).reshape(KN1, 128, KN2, 128).transpose(1, 2, 0, 3), NP_F16)
    w1t = _dt_np(
        W1.astype(NP_F16).reshape(KD, 128, KH, 128).transpose(1, 2, 0, 3), NP_F16)
    w2t = _dt_np(
        W2.astype(NP_F16).reshape(KH, 128, KN2, 128).transpose(1, 2, 0, 3), NP_F16)
    cvec = _dt_np(c.reshape(KN2, 128).T, np.float32)
    ab2 = _dt_np((ALPHA * b2.astype(np.float32)).reshape(KN2, 128).T, np.float32)
    b1t = _dt_np(b1.astype(np.float32).reshape(KH, 128).T, np.float32)

    shared = {"v16": v16, "vtm": vtm, "w1t": w1t, "w2t": w2t,
              "cvec": cvec, "ab2": ab2, "b1t": b1t}

    in_maps = []
    for i in range(NCORES):
        rows = slice(i * BS, (i + 1) * BS)
        dT = np.ascontiguousarray(d[rows].T)        # (D, BS)
        dt16 = _dt_np(dT.reshape(KD, 128, BS).transpose(1, 0, 2), NP_F16)
        z0T = np.ascontiguousarray(z0[rows].T)      # (N2, BS)
        z0t = _dt_np(z0T.reshape(KN2, 128, BS).transpose(1, 0, 2), np.float32)
        in_maps.append({**shared, "dt16": dt16, "z0t": z0t})
    return in_maps


def kernel(d, A, b_vec, W1, b1, W2, b2, z0):
    in_maps = host_in_maps(d, A, b_vec, W1, b1, W2, b2, z0)

    if "nc" not in _CACHE:
        _CACHE["nc"] = _build_program()
    nc = _CACHE["nc"]

    trace = os.environ.get("DYS_TRACE", "0") == "1"
    res = run_bass_kernel_spmd(nc, in_maps, list(range(NCORES)), trace=trace)
    _CACHE["last_result"] = res

    out = np.empty((B, N2), dtype=np.float32)
    for i in range(NCORES):
        arr = res.results[i]["outt"]                # (128, N2/128, BS)
        out[i * BS:(i + 1) * BS] = arr.transpose(2, 1, 0).reshape(BS, N2)
    return out


# revision 6
# speedup vs baseline: 1.6395x; 1.0533x over previous
"""Trainium2 Bass kernel for nn_DYS_opt_net: MLP + 50-iteration DYS fixed-point
loop + one final differentiable step, data-parallel over the batch on 8 cores.

Math (equivalent to the SVD-projector reference):
  P = pinv(A) A = V V^T with V = R^T, R = L^{-1} A, L = chol(A A^T)  (host, fp64)
  c = pinv(A) b = R^T (L^{-1} b)
  w = relu(d W1 + b1) W2 + b2
  51 identical updates:  x = relu(z); u = 2x - z - a*w = |z| - a*w
                         z' = x - a*w + c - (u V) V^T = s - (u V) V^T
  with s = relu(z) + sm, sm = c - a*w (constant across iterations).
  output = relu(z_51)

Device strategy (per core, batch slice of 32 rows, N2-major layout):
  state z^T stored as (128, 32, 32) tiles (partition = N2 % 128).
  mm1:  T^T = V-chunk^T x u^T, k-outer so matmuls consume u16 chunks in the
        order the previous iteration's prep produced them (no boundary stall).
  mm2:  z2^T = V^T-chunks x T^T in fp16 (single pass; fp16 projector error
        ~2e-3 final, well inside the 2e-2 gate), 4 m-groups with alternating
        PSUM tiles; each group's drain + next-iter prep overlaps the next
        group's matmuls on vector/scalar engines.
"""

import os
import sys
from contextlib import ExitStack

import numpy as np

try:
    import concourse.bass as bass
except ImportError:
    sys.path.insert(0, "/opt/trn_rl_repo")
    import concourse.bass as bass

import concourse.tile as tile
from concourse import bacc, mybir
from concourse.bass_utils import run_bass_kernel_spmd

F16 = mybir.dt.float16
F32 = mybir.dt.float32
NP_F16 = np.float16

ALPHA = np.float32(0.05)
NCORES = 8
B, D, H, N1, N2 = 256, 512, 2048, 1024, 4096
BS = B // NCORES          # 32 batch rows per core
ITERS = int(os.environ.get("DYS_ITERS", "51"))   # 50 loop + 1 final step
UNROLL_STATIC = os.environ.get("DYS_UNROLL", "0") == "1"

KD, KH, KN2, KN1 = D // 128, H // 128, N2 // 128, N1 // 128   # 4, 16, 32, 8
G = 4                     # mm2 drain groups
GM = KN2 // G             # m-chunks per group (8)


def _dt_np(x, dt):
    return np.ascontiguousarray(x, dtype=dt)


def _build_program():
    """Build + tile-schedule the SPMD program (same for every core)."""
    nc = bacc.Bacc("TRN2", target_bir_lowering=False, debug=False,
                   num_devices=NCORES)

    dram = {}
    def din(name, shape, dt):
        dram[name] = nc.dram_tensor(name, list(shape), dt, kind="ExternalInput").ap()
    din("dt16", (128, KD, BS), F16)              # d^T slice
    din("b1t", (128, KH), F32)                   # b1
    din("z0t", (128, KN2, BS), F32)              # z0^T slice
    din("cvec", (128, KN2), F32)                 # c
    din("ab2", (128, KN2), F32)                  # alpha*b2
    din("w1t", (128, KH, KD, 128), F16)          # W1 d-major, m-major tiles
    din("w2t", (128, KN2, KH, 128), F16)         # W2 h-major, m-major tiles
    din("v16", (128, KN2, N1), F16)              # V n2-major
    din("vtm", (128, KN2, KN1, 128), F16)        # V^T, m-major tiles
    out_d = nc.dram_tensor("outt", [128, KN2, BS], F32, kind="ExternalOutput").ap()

    with tile.TileContext(nc) as tc:
        with ExitStack() as ctx:
            res = ctx.enter_context(tc.tile_pool(name="resident", bufs=1))
            st = ctx.enter_context(tc.tile_pool(name="state", bufs=1))
            tmp = ctx.enter_context(tc.tile_pool(name="tmp", bufs=1))
            w2p = ctx.enter_context(tc.tile_pool(name="w2s", bufs=3))
            ps = ctx.enter_context(tc.tile_pool(name="ps", bufs=1, space="PSUM"))

            # ---- small front-loaded DMAs (everything the MLP + prologue needs) ----
            dt16 = st.tile([128, KD, BS], F16)
            b1t = st.tile([128, KH], F32)
            zt = st.tile([128, KN2, BS], F32)
            cv = st.tile([128, KN2], F32)
            ab2 = st.tile([128, KN2], F32)
            nc.sync.dma_start(dt16[:], dram["dt16"][:])
            nc.sync.dma_start(b1t[:], dram["b1t"][:])
            nc.sync.dma_start(zt[:], dram["z0t"][:])
            nc.sync.dma_start(cv[:], dram["cvec"][:])
            nc.sync.dma_start(ab2[:], dram["ab2"][:])

            # ---- MLP: w^T = W2^T relu(W1^T d^T + b1) + b2, then aw = a*w^T ----
            ht_ps = ps.tile([128, KH, BS], F32)
            for m in range(KH):
                w1tile = w2p.tile([128, KD, 128], F16, tag="w1t")
                nc.sync.dma_start(w1tile[:], dram["w1t"][:, m])
                for k in range(KD):
                    nc.tensor.matmul(ht_ps[:, m, :], w1tile[:, k, :],
                                     dt16[:, k, :], start=(k == 0), stop=(k == KD - 1))
            hadd = tmp.tile([128, KH, BS], F32, tag="a")
            nc.vector.tensor_add(hadd[:], ht_ps[:], b1t[:, :, None].to_broadcast((128, KH, BS)))
            ht16 = st.tile([128, KH, BS], F16)
            nc.scalar.activation(ht16[:], hadd[:], mybir.ActivationFunctionType.Relu)

            wt_ps = ps.tile([128, KN2, BS], F32)
            for m in range(KN2):
                w2tile = w2p.tile([128, KH, 128], F16, tag="w2t")
                nc.sync.dma_start(w2tile[:], dram["w2t"][:, m])
                for k in range(KH):
                    nc.tensor.matmul(wt_ps[:, m, :], w2tile[:, k, :], ht16[:, k, :],
                                     start=(k == 0), stop=(k == KH - 1))
            aw = st.tile([128, KN2, BS], F32)
            sm = st.tile([128, KN2, BS], F32)
            nc.vector.tensor_scalar_mul(aw[:], wt_ps[:], float(ALPHA))
            nc.vector.tensor_add(aw[:], aw[:], ab2[:, :, None].to_broadcast((128, KN2, BS)))
            # sm = c - aw
            nc.vector.tensor_scalar_mul(sm[:], aw[:], -1.0)
            nc.vector.tensor_add(sm[:], sm[:], cv[:, :, None].to_broadcast((128, KN2, BS)))

            # ---- resident V weights (queued after MLP weights; loop chases) ----
            v16 = res.tile([128, KN2, N1], F16)
            vtm = res.tile([128, KN2, KN1, 128], F16)
            for c8 in range(0, KN2, 8):
                nc.sync.dma_start(v16[:, c8:c8 + 8, :], dram["v16"][:, c8:c8 + 8, :])
            for c8 in range(0, KN2, 8):
                nc.sync.dma_start(vtm[:, c8:c8 + 8], dram["vtm"][:, c8:c8 + 8])

            # ---- state + prologue prep from z0 ----
            u16 = st.tile([128, KN2, BS], F16)
            s = st.tile([128, KN2, BS], F32)
            tt16 = st.tile([128, KN1, BS], F16)
            ab_t = tmp.tile([128, KN2, BS], F32, tag="a")
            rl_t = tmp.tile([128, KN2, BS], F32, tag="r")

            def prep(gsl):
                """From fresh zt[gsl]: u16 = |z| - aw, s = relu(z) + sm."""
                n = gsl.stop - gsl.start
                nc.scalar.activation(ab_t[:, gsl], zt[:, gsl],
                                     mybir.ActivationFunctionType.Abs)
                nc.vector.tensor_sub(u16[:, gsl], ab_t[:, gsl], aw[:, gsl])
                nc.scalar.activation(rl_t[:, gsl], zt[:, gsl],
                                     mybir.ActivationFunctionType.Relu)
                nc.vector.tensor_add(s[:, gsl], rl_t[:, gsl], sm[:, gsl])

            prep(slice(0, KN2))

            # ---- DYS iterations ----
            tt_ps = ps.tile([128, KN1, BS], F32)
            z2a = ps.tile([128, GM, BS], F32)
            z2b = ps.tile([128, GM, BS], F32)
            z2ps = [z2a, z2b]

            def body(_i):
                # mm1: T^T += V-chunk^T @ u^T (one PSUM group open per bank:
                # m-outer, k-inner; first m-chunk consumes u16 in k order).
                # tt16 copies interleave as m-chunks complete, alternating
                # scalar/vector so both halves are done by mm1's end.
                for m in range(KN1):
                    for k in range(KN2):
                        nc.tensor.matmul(tt_ps[:, m, :],
                                         v16[:, k, m * 128:(m + 1) * 128],
                                         u16[:, k, :],
                                         start=(k == 0), stop=(k == KN2 - 1))
                    if m % 2 == 1:
                        csl = slice(m - 1, m + 1)
                        if m % 4 == 1:
                            nc.scalar.activation(tt16[:, csl], tt_ps[:, csl],
                                                 mybir.ActivationFunctionType.Copy)
                        else:
                            nc.vector.tensor_scalar_mul(tt16[:, csl], tt_ps[:, csl], 1.0)
                # mm2 in G groups; drain + next-iter prep overlap next group
                for g in range(G):
                    zp = z2ps[g % 2]
                    base = g * GM
                    for mo in range(GM):
                        for k in range(KN1):
                            nc.tensor.matmul(zp[:, mo, :],
                                             vtm[:, base + mo, k, :],
                                             tt16[:, k, :],
                                             start=(k == 0), stop=(k == KN1 - 1))
                    gsl = slice(base, base + GM)
                    nc.vector.tensor_sub(zt[:, gsl], s[:, gsl], zp[:])
                    prep(gsl)

            if UNROLL_STATIC:
                for i in range(ITERS):
                    body(i)
            else:
                with tc.For_i(0, ITERS, 1, hint_engines=(mybir.EngineType.PE,)) as i:
                    body(i)

            # ---- output: relu(z) ----
            outs = tmp.tile([128, KN2, BS], F32, tag="r")
            nc.scalar.activation(outs[:], zt[:], mybir.ActivationFunctionType.Relu)
            nc.sync.dma_start(out_d[:], outs[:])

    nc.compile()
    return nc


_CACHE = {}


def _host_factors(A, b_vec):
    A64 = A.astype(np.float64)
    L = np.linalg.cholesky(A64 @ A64.T)
    R = np.linalg.solve(L, A64)                     # (N1, N2), orthonormal rows
    q = np.linalg.solve(L, b_vec.astype(np.float64))
    c = (R.T @ q).astype(np.float32)                # (N2,)
    VT = R.astype(np.float32)                       # (N1, N2) = V^T
    V = np.ascontiguousarray(VT.T)                  # (N2, N1)
    return V, VT, c


def host_in_maps(d, A, b_vec, W1, b1, W2, b2, z0):
    """Host-side factor computation + per-core DRAM layouts."""
    V, VT, c = _host_factors(A, b_vec)

    v16 = _dt_np(V.reshape(KN2, 128, N1).transpose(1, 0, 2), NP_F16)
    # (N1, N2) -> (128, m=N2/128, k=N1/128, 128)
    vtm = _dt_np(
        VT.astype(NP_F16).reshape(KN1, 128, KN2, 128).transpose(1, 2, 0, 3), NP_F16)
    w1t = _dt_np(
        W1.astype(NP_F16).reshape(KD, 128, KH, 128).transpose(1, 2, 0, 3), NP_F16)
    w2t = _dt_np(
        W2.astype(NP_F16).reshape(KH, 128, KN2, 128).transpose(1, 2, 0, 3), NP_F16)
    cvec = _dt_np(c.reshape(KN2, 128).T, np.float32)
    ab2 = _dt_np((ALPHA * b2.astype(np.float32)).reshape(KN2, 128).T, np.float32)
    b1t = _dt_np(b1.astype(np.float32).reshape(KH, 128).T, np.float32)

    shared = {"v16": v16, "vtm": vtm, "w1t": w1t, "w2t": w2t,
              "cvec": cvec, "ab2": ab2, "b1t": b1t}

    in_maps = []
    for i in range(NCORES):
        rows = slice(i * BS, (i + 1) * BS)
        dT = np.ascontiguousarray(d[rows].T)        # (D, BS)
        dt16 = _dt_np(dT.reshape(KD, 128, BS).transpose(1, 0, 2), NP_F16)
        z0T = np.ascontiguousarray(z0[rows].T)      # (N2, BS)
        z0t = _dt_np(z0T.reshape(KN2, 128, BS).transpose(1, 0, 2), np.float32)
        in_maps.append({**shared, "dt16": dt16, "z0t": z0t})
    return in_maps


def kernel(d, A, b_vec, W1, b1, W2, b2, z0):
    in_maps = host_in_maps(d, A, b_vec, W1, b1, W2, b2, z0)

    if "nc" not in _CACHE:
        _CACHE["nc"] = _build_program()
    nc = _CACHE["nc"]

    trace = os.environ.get("DYS_TRACE", "0") == "1"
    res = run_bass_kernel_spmd(nc, in_maps, list(range(NCORES)), trace=trace)
    _CACHE["last_result"] = res

    out = np.empty((B, N2), dtype=np.float32)
    for i in range(NCORES):
        arr = res.results[i]["outt"]                # (128, N2/128, BS)
        out[i * BS:(i + 1) * BS] = arr.transpose(2, 1, 0).reshape(BS, N2)
    return out


# revision 9
# speedup vs baseline: 1.8521x; 1.1297x over previous
"""Trainium2 Bass kernel for nn_DYS_opt_net: MLP + 50-iteration DYS fixed-point
loop + one final differentiable step, data-parallel over the batch on 8 cores.

Math (equivalent to the SVD-projector reference):
  P = pinv(A) A = V V^T with V = R^T, R = L^{-1} A, L = chol(A A^T)  (host, fp64)
  c = pinv(A) b = R^T (L^{-1} b)
  w = relu(d W1 + b1) W2 + b2
  51 identical updates:  x = relu(z); u = 2x - z - a*w = |z| - a*w
                         z' = x - a*w + c - (u V) V^T = s - (u V) V^T
  with s = relu(z) + sm, sm = c - a*w (constant across iterations).
  output = relu(z_51)

Device strategy (per core, batch slice of 32 rows, N2-major layout):
  state z^T stored as (128, 32, 32) tiles (partition = N2 % 128).
  mm1:  T^T = V-chunk^T x u^T, k-outer so matmuls consume u16 chunks in the
        order the previous iteration's prep produced them (no boundary stall).
  mm2:  z2^T = V^T-chunks x T^T in fp16 (single pass; fp16 projector error
        ~2e-3 final, well inside the 2e-2 gate), 4 m-groups with alternating
        PSUM tiles; each group's drain + next-iter prep overlaps the next
        group's matmuls on vector/scalar engines.
"""

import os
import sys
from contextlib import ExitStack

import numpy as np

try:
    import concourse.bass as bass
except ImportError:
    sys.path.insert(0, "/opt/trn_rl_repo")
    import concourse.bass as bass

import concourse.tile as tile
from concourse import bacc, mybir
from concourse.bass_utils import run_bass_kernel_spmd

F16 = mybir.dt.float16
F32 = mybir.dt.float32
NP_F16 = np.float16

ALPHA = np.float32(0.05)
NCORES = 8
B, D, H, N1, N2 = 256, 512, 2048, 1024, 4096
BS = B // NCORES          # 32 batch rows per core
ITERS = int(os.environ.get("DYS_ITERS", "51"))   # 50 loop + 1 final step
UNROLL_STATIC = os.environ.get("DYS_UNROLL", "0") == "1"

KD, KH, KN2, KN1 = D // 128, H // 128, N2 // 128, N1 // 128   # 4, 16, 32, 8
G = 4                     # mm2 drain groups
GM = KN2 // G             # m-chunks per group (8)


def _dt_np(x, dt):
    return np.ascontiguousarray(x, dtype=dt)


def _build_program():
    """Build + tile-schedule the SPMD program (same for every core)."""
    nc = bacc.Bacc("TRN2", target_bir_lowering=False, debug=False,
                   num_devices=NCORES)

    dram = {}
    def din(name, shape, dt):
        dram[name] = nc.dram_tensor(name, list(shape), dt, kind="ExternalInput").ap()
    din("dt16", (128, KD, BS), F16)              # d^T slice
    din("b1t", (128, KH), F32)                   # b1
    din("z0t", (128, KN2, BS), F32)              # z0^T slice
    din("cvec", (128, KN2), F32)                 # c
    din("ab2", (128, KN2), F32)                  # alpha*b2
    din("w1t", (128, KH, KD, 128), F16)          # W1 d-major, m-major tiles
    din("w2t", (128, KN2, KH, 128), F16)         # W2 h-major, m-major tiles
    din("v16", (128, KN2, N1), F16)              # V n2-major
    din("vtm", (128, KN2, KN1, 128), F16)        # V^T, m-major tiles
    out_d = nc.dram_tensor("outt", [128, KN2, BS], F32, kind="ExternalOutput").ap()

    with tile.TileContext(nc) as tc:
        with ExitStack() as ctx:
            res = ctx.enter_context(tc.tile_pool(name="resident", bufs=1))
            st = ctx.enter_context(tc.tile_pool(name="state", bufs=1))
            tmp = ctx.enter_context(tc.tile_pool(name="tmp", bufs=1))
            w2p = ctx.enter_context(tc.tile_pool(name="w2s", bufs=3))
            ps = ctx.enter_context(tc.tile_pool(name="ps", bufs=1, space="PSUM"))

            # ---- small front-loaded DMAs (everything the MLP + prologue needs) ----
            dt16 = st.tile([128, KD, BS], F16)
            b1t = st.tile([128, KH], F32)
            zt = st.tile([128, KN2, BS], F32)
            cv = st.tile([128, KN2], F32)
            ab2 = st.tile([128, KN2], F32)
            nc.sync.dma_start(dt16[:], dram["dt16"][:])
            nc.sync.dma_start(b1t[:], dram["b1t"][:])
            nc.sync.dma_start(zt[:], dram["z0t"][:])
            nc.sync.dma_start(cv[:], dram["cvec"][:])
            nc.sync.dma_start(ab2[:], dram["ab2"][:])

            # ---- MLP: w^T = W2^T relu(W1^T d^T + b1) + b2, then aw = a*w^T ----
            ht_ps = ps.tile([128, KH, BS], F32)
            for m in range(KH):
                w1tile = w2p.tile([128, KD, 128], F16, tag="w1t")
                nc.sync.dma_start(w1tile[:], dram["w1t"][:, m])
                for k in range(KD):
                    nc.tensor.matmul(ht_ps[:, m, :], w1tile[:, k, :],
                                     dt16[:, k, :], start=(k == 0), stop=(k == KD - 1))
            hadd = tmp.tile([128, KH, BS], F32, tag="a")
            nc.vector.tensor_add(hadd[:], ht_ps[:], b1t[:, :, None].to_broadcast((128, KH, BS)))
            ht16 = st.tile([128, KH, BS], F16)
            nc.scalar.activation(ht16[:], hadd[:], mybir.ActivationFunctionType.Relu)

            wt_ps = ps.tile([128, KN2, BS], F32)
            for m in range(KN2):
                w2tile = w2p.tile([128, KH, 128], F16, tag="w2t")
                nc.sync.dma_start(w2tile[:], dram["w2t"][:, m])
                for k in range(KH):
                    nc.tensor.matmul(wt_ps[:, m, :], w2tile[:, k, :], ht16[:, k, :],
                                     start=(k == 0), stop=(k == KH - 1))
            aw = st.tile([128, KN2, BS], F32)
            sm = st.tile([128, KN2, BS], F32)
            nc.vector.tensor_scalar_mul(aw[:], wt_ps[:], float(ALPHA))
            nc.vector.tensor_add(aw[:], aw[:], ab2[:, :, None].to_broadcast((128, KN2, BS)))
            # sm = c - aw
            nc.vector.tensor_scalar_mul(sm[:], aw[:], -1.0)
            nc.vector.tensor_add(sm[:], sm[:], cv[:, :, None].to_broadcast((128, KN2, BS)))

            # ---- resident V weights (queued after MLP weights; loop chases) ----
            v16 = res.tile([128, KN2, N1], F16)
            vtm = res.tile([128, KN2, KN1, 128], F16)
            for c8 in range(0, KN2, 8):
                nc.sync.dma_start(v16[:, c8:c8 + 8, :], dram["v16"][:, c8:c8 + 8, :])
            for c8 in range(0, KN2, 8):
                nc.sync.dma_start(vtm[:, c8:c8 + 8], dram["vtm"][:, c8:c8 + 8])

            # ---- state + prologue prep from z0 ----
            u16 = st.tile([128, KN2, BS], F16)
            s = st.tile([128, KN2, BS], F32)
            tt16 = st.tile([128, KN1, BS], F16)
            ab_t = tmp.tile([128, KN2, BS], F32, tag="a")
            rl_t = tmp.tile([128, KN2, BS], F32, tag="r")

            def prep(gsl):
                """From fresh zt[gsl]: u16 = |z| - aw, s = relu(z) + sm."""
                n = gsl.stop - gsl.start
                nc.scalar.activation(ab_t[:, gsl], zt[:, gsl],
                                     mybir.ActivationFunctionType.Abs)
                nc.vector.tensor_sub(u16[:, gsl], ab_t[:, gsl], aw[:, gsl])
                nc.scalar.activation(rl_t[:, gsl], zt[:, gsl],
                                     mybir.ActivationFunctionType.Relu)
                nc.vector.tensor_add(s[:, gsl], rl_t[:, gsl], sm[:, gsl])

            prep(slice(0, KN2))

            # ---- DYS iterations ----
            # each accumulation tile padded to a full 2KB PSUM bank so the
            # one-open-group-per-bank rule never couples unrelated tiles
            tt_psa = ps.tile([128, KN1 // 2, BS], F32, padded_shape=(None, 16, None))
            tt_psb = ps.tile([128, KN1 // 2, BS], F32, padded_shape=(None, 16, None))
            z2a = ps.tile([128, GM, BS], F32, padded_shape=(None, 16, None))
            z2b = ps.tile([128, GM, BS], F32, padded_shape=(None, 16, None))
            z2ps = [z2a, z2b]

            def mm1_run(m, ks):
                """One run of mm1 k-accumulation for output chunk m."""
                tp = tt_psa if m < 4 else tt_psb
                mi = m % 4
                for k in ks:
                    nc.tensor.matmul(tp[:, mi, :],
                                     v16[:, k, m * 128:(m + 1) * 128],
                                     u16[:, k, :],
                                     start=(k == 0), stop=(k == KN2 - 1))

            # mm2 contracts k in this order so the last-copied tt16 chunks
            # (2,3 — from the last-closing mm1 runs m2,m3) are read last
            KORD = [0, 1, 4, 5, 6, 7, 2, 3]

            def body(_i):
                # mm1: T^T += V-chunk^T @ u^T. Two PSUM banks (m0-3 / m4-7)
                # let m0 split so its k24..31 (group-3 u16, the last chunks
                # prepped by the previous iteration) are read ~1.5us in.
                mm1_run(0, range(0, KN2))
                mm1_run(4, range(0, KN2))
                mm1_run(5, range(0, KN2))
                nc.scalar.activation(tt16[:, 4:6], tt_psb[:, 0:2],
                                     mybir.ActivationFunctionType.Copy)
                mm1_run(1, range(0, KN2))
                nc.vector.tensor_scalar_mul(tt16[:, 0:2], tt_psa[:, 0:2], 1.0)
                mm1_run(6, range(0, KN2))
                mm1_run(2, range(0, KN2))
                mm1_run(7, range(0, KN2))
                nc.scalar.activation(tt16[:, 6:8], tt_psb[:, 2:4],
                                     mybir.ActivationFunctionType.Copy)
                mm1_run(3, range(0, KN2))
                nc.vector.tensor_scalar_mul(tt16[:, 2:4], tt_psa[:, 2:4], 1.0)
                # mm2 in G groups; drain + next-iter prep overlap next group
                for g in range(G):
                    zp = z2ps[g % 2]
                    base = g * GM
                    for mo in range(GM):
                        for j, k in enumerate(KORD):
                            nc.tensor.matmul(zp[:, mo, :],
                                             vtm[:, base + mo, k, :],
                                             tt16[:, k, :],
                                             start=(j == 0), stop=(j == KN1 - 1))
                    gsl = slice(base, base + GM)
                    nc.vector.tensor_sub(zt[:, gsl], s[:, gsl], zp[:])
                    prep(gsl)

            if UNROLL_STATIC:
                for i in range(ITERS):
                    body(i)
            else:
                with tc.For_i(0, ITERS, 1, hint_engines=(mybir.EngineType.PE,)) as i:
                    body(i)

            # ---- output: relu(z) ----
            outs = tmp.tile([128, KN2, BS], F32, tag="r")
            nc.scalar.activation(outs[:], zt[:], mybir.ActivationFunctionType.Relu)
            nc.sync.dma_start(out_d[:], outs[:])

    nc.compile()
    return nc


_CACHE = {}


def _host_factors(A, b_vec):
    A64 = A.astype(np.float64)
    L = np.linalg.cholesky(A64 @ A64.T)
    R = np.linalg.solve(L, A64)                     # (N1, N2), orthonormal rows
    q = np.linalg.solve(L, b_vec.astype(np.float64))
    c = (R.T @ q).astype(np.float32)                # (N2,)
    VT = R.astype(np.float32)                       # (N1, N2) = V^T
    V = np.ascontiguousarray(VT.T)                  # (N2, N1)
    return V, VT, c


def host_in_maps(d, A, b_vec, W1, b1, W2, b2, z0):
    """Host-side factor computation + per-core DRAM layouts."""
    V, VT, c = _host_factors(A, b_vec)

    v16 = _dt_np(V.reshape(KN2, 128, N1).transpose(1, 0, 2), NP_F16)
    # (N1, N2) -> (128, m=N2/128, k=N1/128, 128)
    vtm = _dt_np(
        VT.astype(NP_F16).reshape(KN1, 128, KN2, 128).transpose(1, 2, 0, 3), NP_F16)
    w1t = _dt_np(
        W1.astype(NP_F16).reshape(KD, 128, KH, 128).transpose(1, 2, 0, 3), NP_F16)
    w2t = _dt_np(
        W2.astype(NP_F16).reshape(KH, 128, KN2, 128).transpose(1, 2, 0, 3), NP_F16)
    cvec = _dt_np(c.reshape(KN2, 128).T, np.float32)
    ab2 = _dt_np((ALPHA * b2.astype(np.float32)).reshape(KN2, 128).T, np.float32)
    b1t = _dt_np(b1.astype(np.float32).reshape(KH, 128).T, np.float32)

    shared = {"v16": v16, "vtm": vtm, "w1t": w1t, "w2t": w2t,
              "cvec": cvec, "ab2": ab2, "b1t": b1t}

    in_maps = []
    for i in range(NCORES):
        rows = slice(i * BS, (i + 1) * BS)
        dT = np.ascontiguousarray(d[rows].T)        # (D, BS)
        dt16 = _dt_np(dT.reshape(KD, 128, BS).transpose(1, 0, 2), NP_F16)
        z0T = np.ascontiguousarray(z0[rows].T)      # (N2, BS)
        z0t = _dt_np(z0T.reshape(KN2, 128, BS).transpose(1, 0, 2), np.float32)
        in_maps.append({**shared, "dt16": dt16, "z0t": z0t})
    return in_maps


def kernel(d, A, b_vec, W1, b1, W2, b2, z0):
    in_maps = host_in_maps(d, A, b_vec, W1, b1, W2, b2, z0)

    if "nc" not in _CACHE:
        _CACHE["nc"] = _build_program()
    nc = _CACHE["nc"]

    trace = os.environ.get("DYS_TRACE", "0") == "1"
    res = run_bass_kernel_spmd(nc, in_maps, list(range(NCORES)), trace=trace)
    _CACHE["last_result"] = res

    out = np.empty((B, N2), dtype=np.float32)
    for i in range(NCORES):
        arr = res.results[i]["outt"]                # (128, N2/128, BS)
        out[i * BS:(i + 1) * BS] = arr.transpose(2, 1, 0).reshape(BS, N2)
    return out
